# revision 1
# baseline (speedup 1.0000x reference)
"""Trainium2 Bass kernel for the Mamba-style selective-scan block
(nn_Block_24962349924931).

Shapes: x [2, 4096, 1024]; D_MODEL=1024, D_INNER=2048, D_STATE=16, K=3.

Sharding: 8 cores = DP2 (batch) x TP4 (d_inner channels, 512/core).
Two device launches with a host-side exchange of u between them:

  Launch A: u = silu(im2col(x) @ W_fused + b_fused)   [per-core channel shard]
            where W_fused = in_w @ conv_w (conv + in_proj fused on host; the
            conv output xc is never materialized).
  host: gather u shards -> full u per batch.
  Launch B: delta = softplus(u @ dt_w^T + dt_b); B/C projections emitted
            directly in a row-replicated (e_sub, n) partition layout;
            selective scan via HW tensor_tensor_scan (state on partitions,
            time on the free dim); y = sum_n C*h via selector matmuls into
            an accumulating PSUM tile; out_partial = (y + u*Dskip) @ out_w^T.
  host: sum the 4 TP partials per batch, add out_b, stack batches.
"""
import sys
sys.path.insert(0, "/opt/trn_rl_repo")

import numpy as np
import ml_dtypes

import concourse.bacc as bacc
import concourse.mybir as mybir
from concourse import bass_utils
from concourse.tile import TileContext

F32 = mybir.dt.float32
BF16 = mybir.dt.bfloat16
AL = mybir.AluOpType
AF = mybir.ActivationFunctionType
BF16NP = ml_dtypes.bfloat16

# ---- problem constants ----
B, L, D, E, N, K = 2, 4096, 1024, 2048, 16, 3
NCORES = 8
TPG = 4              # cores per batch (tensor parallel group)
EL = E // TPG        # 512 channels per core
FT = EL // 128       # 4 f-tiles per core
KT_U = E // 128      # 16 k-tiles over full E
KT_X = (D * K) // 128  # 24 k-tiles over im2col contraction
TC = L // 512        # 8 t-chunks
JN = 16              # rep-tiles per f-tile (8 channels x 16 states each)

# timing loop reps (0 = single shot); test.py rebuilds with reps>0
_LOOP_REPS = 0


def _bf16(a):
    return np.ascontiguousarray(np.asarray(a).astype(BF16NP))


# ===================================================================
# Launch A: u = silu(im2col(x) @ W_fused + b_fused)
# ===================================================================
def build_launch_a(loop_reps=0):
    nc = bacc.Bacc("TRN2", target_bir_lowering=False, debug=False)
    xT = nc.dram_tensor("xT", [D, L + 2], BF16, kind="ExternalInput")
    wf = nc.dram_tensor("wf", [KT_X, 128, EL], BF16, kind="ExternalInput")
    bfu = nc.dram_tensor("bfu", [128, FT], F32, kind="ExternalInput")
    u_out = nc.dram_tensor("u_out", [EL, L], BF16, kind="ExternalOutput")

    with TileContext(nc) as tc:
        if loop_reps:
            tc.race_detector_enabled = False
        with (
            tc.tile_pool(name="big", bufs=1) as big,
            tc.tile_pool(name="work", bufs=4) as work,
            tc.tile_pool(name="ps", bufs=8, space="PSUM") as ps,
        ):
            xsb = big.tile([128, 8 * (L + 2)], BF16, tag="xsb")
            for j in range(8):
                nc.sync.dma_start(
                    xsb[:, j * (L + 2):(j + 1) * (L + 2)],
                    xT[j * 128:(j + 1) * 128, :])
            wfsb = big.tile([128, KT_X * EL], BF16, tag="wfsb")
            for kt in range(KT_X):
                nc.sync.dma_start(wfsb[:, kt * EL:(kt + 1) * EL], wf[kt, :, :])
            bfu_t = big.tile([128, FT], F32, tag="bfu")
            nc.sync.dma_start(bfu_t[:, :], bfu[:, :])

            def body(_=None, unroll=None):
                for ft in range(FT):
                    for tcI in range(TC):
                        t0 = tcI * 512
                        pt = ps.tile([128, 512], F32, tag="acc")
                        for kt in range(KT_X):
                            kap, j = divmod(kt, 8)
                            rhs = xsb[:, j * (L + 2) + t0 + kap:
                                      j * (L + 2) + t0 + kap + 512]
                            lhsT = wfsb[:, kt * EL + ft * 128:
                                        kt * EL + (ft + 1) * 128]
                            nc.tensor.matmul(pt[:, :], lhsT, rhs,
                                             start=(kt == 0),
                                             stop=(kt == KT_X - 1))
                        ut = work.tile([128, 512], BF16, tag="u")
                        nc.scalar.activation(ut[:, :], pt[:, :], AF.Silu,
                                             bias=bfu_t[:, ft:ft + 1])
                        nc.sync.dma_start(
                            u_out[ft * 128:(ft + 1) * 128, t0:t0 + 512],
                            ut[:, :])

            if loop_reps:
                with tc.For_i(0, loop_reps, 1) as _i:
                    body()
            else:
                body()
    nc.compile()
    return nc


# ===================================================================
# Launch B: projections + scan + out-proj partial
# ===================================================================
def build_launch_b(loop_reps=0):
    nc = bacc.Bacc("TRN2", target_bir_lowering=False, debug=False)
    u_all = nc.dram_tensor("u_all", [KT_U, 128, L], BF16, kind="ExternalInput")
    dtw = nc.dram_tensor("dtw", [KT_U, 128, EL], BF16, kind="ExternalInput")
    wb = nc.dram_tensor("wb", [KT_U, 128, 128], BF16, kind="ExternalInput")
    wc = nc.dram_tensor("wc", [KT_U, 128, 128], BF16, kind="ExternalInput")
    dtb = nc.dram_tensor("dtb", [128, FT], F32, kind="ExternalInput")
    bpb = nc.dram_tensor("bpb", [128, 1], F32, kind="ExternalInput")
    cpb = nc.dram_tensor("cpb", [128, 1], F32, kind="ExternalInput")
    acol = nc.dram_tensor("acol", [128, FT * JN], F32, kind="ExternalInput")
    sel16 = nc.dram_tensor("sel16", [JN, 128, 128], BF16, kind="ExternalInput")
    sel2g = nc.dram_tensor("sel2g", [JN, 128, 128], BF16, kind="ExternalInput")
    dsk = nc.dram_tensor("dsk", [128, FT], F32, kind="ExternalInput")
    ow = nc.dram_tensor("ow", [FT, 128, 8 * 128], BF16, kind="ExternalInput")
    u_own_in = nc.dram_tensor("u_own", [FT, 128, L], BF16,
                              kind="ExternalInput")
    yp = nc.dram_tensor("yp", [D, L], F32, kind="ExternalOutput")

    with TileContext(nc) as tc:
        if loop_reps:
            tc.race_detector_enabled = False
        with (
            tc.tile_pool(name="big", bufs=1) as big,
            tc.tile_pool(name="work", bufs=3) as work,
            tc.tile_pool(name="ps", bufs=1, space="PSUM") as ps,
        ):
            # ---- resident weights/constants ----
            dtw_t = big.tile([128, KT_U * EL], BF16, tag="dtw")
            wb_t = big.tile([128, KT_U * 128], BF16, tag="wb")
            wc_t = big.tile([128, KT_U * 128], BF16, tag="wc")
            for kt in range(KT_U):
                nc.sync.dma_start(dtw_t[:, kt * EL:(kt + 1) * EL], dtw[kt, :, :])
                nc.sync.dma_start(wb_t[:, kt * 128:(kt + 1) * 128], wb[kt, :, :])
                nc.sync.dma_start(wc_t[:, kt * 128:(kt + 1) * 128], wc[kt, :, :])
            sel16_t = big.tile([128, JN * 128], BF16, tag="sel16")
            sel2g_t = big.tile([128, JN * 128], BF16, tag="sel2g")
            for g in range(JN):
                nc.sync.dma_start(sel16_t[:, g * 128:(g + 1) * 128],
                                  sel16[g, :, :])
                nc.sync.dma_start(sel2g_t[:, g * 128:(g + 1) * 128],
                                  sel2g[g, :, :])
            ow_t = big.tile([128, FT * 8 * 128], BF16, tag="ow")
            for ft in range(FT):
                nc.sync.dma_start(ow_t[:, ft * 1024:(ft + 1) * 1024],
                                  ow[ft, :, :])
            dtb_t = big.tile([128, FT], F32, tag="dtb")
            bpb_t = big.tile([128, 1], F32, tag="bpb")
            cpb_t = big.tile([128, 1], F32, tag="cpb")
            acol_t = big.tile([128, FT * JN], F32, tag="acol")
            dsk_t = big.tile([128, FT], F32, tag="dsk")
            nc.sync.dma_start(dtb_t[:, :], dtb[:, :])
            nc.sync.dma_start(bpb_t[:, :], bpb[:, :])
            nc.sync.dma_start(cpb_t[:, :], cpb[:, :])
            nc.sync.dma_start(acol_t[:, :], acol[:, :])
            nc.sync.dma_start(dsk_t[:, :], dsk[:, :])

            # ---- resident activations ----
            delta_t = big.tile([128, FT * L], BF16, tag="delta")
            brep_t = big.tile([128, L], BF16, tag="brep")
            crep_t = big.tile([128, L], BF16, tag="crep")
            y_t = big.tile([128, FT * L], BF16, tag="y")
            hlast = big.tile([128, FT * JN], F32, tag="hlast")

            def body(_=None, unroll=None):
                # ---- stage 1: dt/B/C projections, w = delta*u_own ----
                for tcI in range(TC):
                    t0 = tcI * 512
                    psD = [ps.tile([128, 512], F32, tag=f"psD{f}",
                                   name=f"psD{f}")
                           for f in range(FT)]
                    psB = ps.tile([128, 512], F32, tag="psB")
                    psC = ps.tile([128, 512], F32, tag="psC")
                    u_own = [None] * FT
                    for kt in range(KT_U):
                        utile = work.tile([128, 512], BF16, tag="u_in")
                        nc.sync.dma_start(utile[:, :], u_all[kt, :, t0:t0 + 512])
                        for ft in range(FT):
                            nc.tensor.matmul(
                                psD[ft][:, :],
                                dtw_t[:, kt * EL + ft * 128:
                                      kt * EL + (ft + 1) * 128],
                                utile[:, :], start=(kt == 0),
                                stop=(kt == KT_U - 1))
                        nc.tensor.matmul(psB[:, :],
                                         wb_t[:, kt * 128:(kt + 1) * 128],
                                         utile[:, :], start=(kt == 0),
                                         stop=(kt == KT_U - 1))
                        nc.tensor.matmul(psC[:, :],
                                         wc_t[:, kt * 128:(kt + 1) * 128],
                                         utile[:, :], start=(kt == 0),
                                         stop=(kt == KT_U - 1))
                    # softplus(z+b) = ln(1 + e^(z+b)); Softplus shares no ACT
                    # table-set with Exp, but Ln+Exp coexist in one set.
                    for ft in range(FT):
                        q1 = work.tile([128, 512], F32, tag="sp_q")
                        nc.scalar.activation(q1[:, :], psD[ft][:, :], AF.Exp,
                                             bias=dtb_t[:, ft:ft + 1])
                        nc.vector.tensor_scalar_add(q1[:, :], q1[:, :], 1.0)
                        nc.scalar.activation(
                            delta_t[:, ft * L + t0: ft * L + t0 + 512],
                            q1[:, :], AF.Ln)
                    nc.scalar.activation(brep_t[:, t0:t0 + 512], psB[:, :],
                                         AF.Identity, bias=bpb_t[:, 0:1])
                    nc.scalar.activation(crep_t[:, t0:t0 + 512], psC[:, :],
                                         AF.Identity, bias=cpb_t[:, 0:1])
                # ---- stage 2: scan volume ----
                for tcI in range(TC):
                    t0 = tcI * 512
                    for ft in range(FT):
                        dslice = delta_t[:, ft * L + t0: ft * L + t0 + 512]
                        # w = delta * u_own for this (ft, tc)
                        uo = work.tile([128, 512], BF16, tag="u_own")
                        nc.sync.dma_start(uo[:, :], u_own_in[ft, :, t0:t0 + 512])
                        wloc = work.tile([128, 512], BF16, tag="wloc")
                        nc.vector.scalar_tensor_tensor(
                            wloc[:, :], dslice, 1.0, uo[:, :],
                            AL.mult, AL.mult)
                        psY = ps.tile([128, 512], F32, tag="psD0",
                                      name="psY")
                        for jn in range(JN):
                            j = ft * JN + jn
                            wslice = wloc[:, :]
                            selg = sel16_t[:, jn * 128:(jn + 1) * 128]
                            psR = ps.tile([128, 512], F32, tag="psR")
                            nc.tensor.matmul(psR[:, :], selg, dslice,
                                             start=True, stop=True)
                            dA = work.tile([128, 512], BF16, tag="dA")
                            nc.scalar.activation(dA[:, :], psR[:, :], AF.Exp,
                                                 scale=acol_t[:, j:j + 1])
                            psW = ps.tile([128, 512], F32, tag="psW")
                            nc.tensor.matmul(psW[:, :], selg, wslice,
                                             start=True, stop=True)
                            X = work.tile([128, 512], BF16, tag="X")
                            nc.vector.tensor_tensor(
                                X[:, :], psW[:, :], brep_t[:, t0:t0 + 512],
                                AL.mult)
                            h = work.tile([128, 512], BF16, tag="h")
                            init = 0.0 if tcI == 0 else hlast[:, j:j + 1]
                            nc.vector.tensor_tensor_scan(
                                h[:, :], dA[:, :], X[:, :], init,
                                AL.mult, AL.add)
                            nc.vector.tensor_copy(hlast[:, j:j + 1],
                                                  h[:, 511:512])
                            ch = work.tile([128, 512], BF16, tag="ch")
                            nc.vector.scalar_tensor_tensor(
                                ch[:, :], h[:, :], 1.0,
                                crep_t[:, t0:t0 + 512], AL.mult, AL.mult)
                            nc.tensor.matmul(
                                psY[:, :], sel2g_t[:, jn * 128:(jn + 1) * 128],
                                ch[:, :], start=(jn == 0), stop=(jn == JN - 1))
                        nc.scalar.activation(
                            y_t[:, ft * L + t0: ft * L + t0 + 512],
                            psY[:, :], AF.Copy)

                # ---- stage 3: y_total & out-proj ----
                for tcI in range(TC):
                    t0 = tcI * 512
                    yt = [None] * FT
                    for ft in range(FT):
                        uo = work.tile([128, 512], BF16, tag="u_own3")
                        nc.sync.dma_start(uo[:, :], u_own_in[ft, :, t0:t0 + 512])
                        ytf = work.tile([128, 512], BF16, tag=f"yt{ft}")
                        nc.vector.scalar_tensor_tensor(
                            ytf[:, :], uo[:, :], dsk_t[:, ft:ft + 1],
                            y_t[:, ft * L + t0: ft * L + t0 + 512],
                            AL.mult, AL.add)
                        yt[ft] = ytf
                    for mt in range(8):
                        psO = ps.tile([128, 512], F32, tag="psD1",
                                      name="psO")
                        for ft in range(FT):
                            nc.tensor.matmul(
                                psO[:, :],
                                ow_t[:, ft * 1024 + mt * 128:
                                     ft * 1024 + (mt + 1) * 128],
                                yt[ft][:, :], start=(ft == 0),
                                stop=(ft == FT - 1))
                        ot = work.tile([128, 512], F32, tag="ot")
                        nc.scalar.activation(ot[:, :], psO[:, :], AF.Copy)
                        nc.sync.dma_start(
                            yp[mt * 128:(mt + 1) * 128, t0:t0 + 512], ot[:, :])

            if loop_reps:
                with tc.For_i(0, loop_reps, 1) as _i:
                    body()
            else:
                body()
    nc.compile()
    return nc


# ===================================================================
# Host-side weight preparation
# ===================================================================
def prepare(inputs):
    x = np.asarray(inputs["x"], np.float32)
    conv_w = np.asarray(inputs["conv_w"], np.float32)
    conv_b = np.asarray(inputs["conv_b"], np.float32)
    in_w = np.asarray(inputs["in_w"], np.float32)
    in_b = np.asarray(inputs["in_b"], np.float32)
    A_log = np.asarray(inputs["A_log"], np.float32)
    Dskip = np.asarray(inputs["Dskip"], np.float32)
    dt_w = np.asarray(inputs["dt_w"], np.float32)
    dt_b = np.asarray(inputs["dt_b"], np.float32)
    Bp_w = np.asarray(inputs["Bp_w"], np.float32)
    Bp_b = np.asarray(inputs["Bp_b"], np.float32)
    Cp_w = np.asarray(inputs["Cp_w"], np.float32)
    Cp_b = np.asarray(inputs["Cp_b"], np.float32)
    out_w = np.asarray(inputs["out_w"], np.float32)
    out_b = np.asarray(inputs["out_b"], np.float32)

    # fused conv+in_proj: Wc[f,d,k] = sum_e in_w[f,e] conv_w[e,d,k]
    Wf = (in_w @ conv_w.reshape(E, D * K)).reshape(E, D, K)
    # lhsT rows r = kap*D + d  (kap-major), cols = f
    Wf_knl = Wf.transpose(2, 1, 0).reshape(K * D, E)   # [(kap,d), f]
    b_fused = in_w @ conv_b + in_b                      # [E]

    A = -np.exp(A_log)                                  # [E, N]

    prep = {"A": A}
    # per-batch xT padded
    prep["xT"] = []
    for b in range(B):
        xt = np.zeros((D, L + 2), np.float32)
        xt[:, 1:L + 1] = x[b].T
        prep["xT"].append(_bf16(xt))

    # per-shard tensors
    prep["wf"], prep["bfu"] = [], []
    prep["dtw"], prep["wbk"], prep["wck"] = [], [], []
    prep["dtb"], prep["acol"], prep["dskc"], prep["owk"] = [], [], [], []
    for s in range(TPG):
        Fc = slice(s * EL, (s + 1) * EL)
        prep["wf"].append(_bf16(Wf_knl[:, Fc].reshape(KT_X, 128, EL)))
        prep["bfu"].append(
            np.ascontiguousarray(b_fused[Fc].reshape(FT, 128).T,
                                 dtype=np.float32))
        prep["dtw"].append(_bf16(dt_w[Fc, :].T.reshape(KT_U, 128, EL)))
        prep["dtb"].append(
            np.ascontiguousarray(dt_b[Fc].reshape(FT, 128).T,
                                 dtype=np.float32))
        # acol[p, j] = A[s*512 + 8*j + (p>>4), p & 15]
        ac = np.empty((128, FT * JN), np.float32)
        p = np.arange(128)
        for j in range(FT * JN):
            ac[:, j] = A[s * EL + 8 * j + (p >> 4), p & 15]
        prep["acol"].append(ac)
        prep["dskc"].append(
            np.ascontiguousarray(Dskip[Fc].reshape(FT, 128).T,
                                 dtype=np.float32))
        # out-proj lhsT: ow[ft][p, mt*128+m] = out_w[mt*128+m, s*512+ft*128+p]
        owk = np.empty((FT, 128, 8 * 128), np.float32)
        for ft in range(FT):
            owk[ft] = out_w[:, s * EL + ft * 128:s * EL + (ft + 1) * 128].T
        prep["owk"].append(_bf16(owk))

    # replicated B/C projection weights (same for all cores)
    pidx = np.arange(128)
    wbk = np.empty((KT_U, 128, 128), np.float32)
    wck = np.empty((KT_U, 128, 128), np.float32)
    for kt in range(KT_U):
        wbk[kt] = Bp_w[pidx % 16, :][:, kt * 128:(kt + 1) * 128].T
        wck[kt] = Cp_w[pidx % 16, :][:, kt * 128:(kt + 1) * 128].T
    prep["wbk_g"] = _bf16(wbk)
    prep["wck_g"] = _bf16(wck)
    prep["bpb_g"] = np.ascontiguousarray(
        Bp_b[pidx % 16].reshape(128, 1), dtype=np.float32)
    prep["cpb_g"] = np.ascontiguousarray(
        Cp_b[pidx % 16].reshape(128, 1), dtype=np.float32)

    # selectors
    sel16 = np.zeros((JN, 128, 128), np.float32)
    for g in range(JN):
        for p in range(128):
            sel16[g, 8 * g + (p >> 4), p] = 1.0
    sel2g = np.zeros((JN, 128, 128), np.float32)
    for r in range(JN):
        for k in range(128):
            sel2g[r, k, 8 * r + (k >> 4)] = 1.0
    prep["sel16_g"] = _bf16(sel16)
    prep["sel2g_g"] = _bf16(sel2g)
    prep["out_b"] = out_b
    return prep


# ===================================================================
# Orchestration
# ===================================================================
_CACHE = {}


def _get_kernels(loop_reps=0):
    key = ("k", loop_reps)
    if key not in _CACHE:
        _CACHE[key] = (build_launch_a(loop_reps), build_launch_b(loop_reps))
    return _CACHE[key]


def run_launch_a(nca, prep, **kw):
    in_maps = []
    for c in range(NCORES):
        b, s = divmod(c, TPG)
        in_maps.append(dict(xT=prep["xT"][b], wf=prep["wf"][s],
                            bfu=prep["bfu"][s]))
    res = bass_utils.run_bass_kernel_spmd(nca, in_maps,
                                          core_ids=list(range(NCORES)), **kw)
    return [r["u_out"] for r in res.results]


def run_launch_b(ncb, prep, u_full, **kw):
    in_maps = []
    for c in range(NCORES):
        b, s = divmod(c, TPG)
        ub = u_full[b]                      # [E, L] bf16
        in_maps.append(dict(
            u_all=np.ascontiguousarray(ub.reshape(KT_U, 128, L)),
            u_own=np.ascontiguousarray(
                ub[s * EL:(s + 1) * EL].reshape(FT, 128, L)),
            dtw=prep["dtw"][s], wb=prep["wbk_g"], wc=prep["wck_g"],
            dtb=prep["dtb"][s], bpb=prep["bpb_g"], cpb=prep["cpb_g"],
            acol=prep["acol"][s], sel16=prep["sel16_g"],
            sel2g=prep["sel2g_g"], dsk=prep["dskc"][s], ow=prep["owk"][s],
        ))
    res = bass_utils.run_bass_kernel_spmd(ncb, in_maps,
                                          core_ids=list(range(NCORES)), **kw)
    return [r["yp"] for r in res.results]


def kernel(**inputs):
    prep = prepare(inputs)
    nca, ncb = _get_kernels(_LOOP_REPS)
    u_shards = run_launch_a(nca, prep)          # 8 x [EL, L] bf16
    u_full = []
    for b in range(B):
        u_full.append(np.concatenate(u_shards[b * TPG:(b + 1) * TPG], axis=0))
    yps = run_launch_b(ncb, prep, u_full)       # 8 x [D, L] fp32
    out = np.empty((B, L, D), np.float32)
    for b in range(B):
        acc = yps[b * TPG].astype(np.float32)
        for s in range(1, TPG):
            acc = acc + yps[b * TPG + s]
        out[b] = acc.T + prep["out_b"][None, :]
    return out



# revision 2
# speedup vs baseline: 1.5916x; 1.5916x over previous
"""Trainium2 Bass kernel for the Mamba-style selective-scan block
(nn_Block_24962349924931) — v2.

Shapes: x [2, 4096, 1024]; D_MODEL=1024, D_INNER=2048, D_STATE=16, K=3.

Sharding: 8 cores = DP2 (batch) x TP4 (d_inner channels, 512/core).
Two device launches with a host-side exchange of u between them:

  Launch A: u = silu(im2col(x) @ W_fused + b_fused) per channel shard,
            plus B/C projection partials (contraction over own channels).
  host: gather u shards -> full u per batch; sum B/C partials, add bias,
        replicate rows to the (channel,state) partition layout.
  Launch B: delta = softplus(u @ dt_w^T + dt_b) (exp/ln on ACT);
            dA/w broadcast to the rep layout via selector matmuls;
            X = wrep*brep and ch = h*crep split across DVE+Pool engines;
            selective scan via tensor_tensor_scan on DVE (time halves,
            state carried in hcarry); y contract via accumulating selector
            matmuls; out_partial = (y + u*Dskip) @ out_w^T.
            P1 of the next half is interleaved into P2 of the current one
            (dedicated psP accumulator bank keeps it off P2's PSUM tags).
  host: sum the 4 TP partials per batch, add out_b, stack batches.
"""
import sys
sys.path.insert(0, "/opt/trn_rl_repo")

import numpy as np
import ml_dtypes

import concourse.bacc as bacc
import concourse.mybir as mybir
from concourse import bass_utils
from concourse.tile import TileContext

F32 = mybir.dt.float32
BF16 = mybir.dt.bfloat16
AL = mybir.AluOpType
AF = mybir.ActivationFunctionType
BF16NP = ml_dtypes.bfloat16

# ---- problem constants ----
B, L, D, E, N, K = 2, 4096, 1024, 2048, 16, 3
NCORES = 8
TPG = 4              # cores per batch (tensor parallel group)
EL = E // TPG        # 512 channels per core
FT = EL // 128       # 4 f-tiles per core
KT_U = E // 128      # 16 k-tiles over full E
KT_X = (D * K) // 128  # 24 k-tiles over im2col contraction
TC = L // 512        # 8 t-chunks
JN = 16              # rep-tiles per f-tile (8 channels x 16 states each)
TH = 2               # time halves in launch B
HL = L // TH         # 2048 cols per half
HC = HL // 512       # 4 chunks per half

_LOOP_REPS = 0


def _bf16(a):
    return np.ascontiguousarray(np.asarray(a).astype(BF16NP))


# ===================================================================
# Launch A: u = silu(im2col(x) @ W_fused + b_fused) + B/C partials
# ===================================================================
def build_launch_a(loop_reps=0):
    nc = bacc.Bacc("TRN2", target_bir_lowering=False, debug=False)
    xT = nc.dram_tensor("xT", [D, L + 2], BF16, kind="ExternalInput")
    wf = nc.dram_tensor("wf", [KT_X, 128, EL], BF16, kind="ExternalInput")
    bfu = nc.dram_tensor("bfu", [128, FT], F32, kind="ExternalInput")
    wbc = nc.dram_tensor("wbc", [FT, 128, 32], BF16, kind="ExternalInput")
    u_out = nc.dram_tensor("u_out", [EL, L], BF16, kind="ExternalOutput")
    bc_out = nc.dram_tensor("bc_out", [32, L], F32, kind="ExternalOutput")

    with TileContext(nc) as tc:
        if loop_reps:
            tc.race_detector_enabled = False
        with (
            tc.tile_pool(name="big", bufs=1) as big,
            tc.tile_pool(name="work", bufs=4) as work,
            tc.tile_pool(name="ps", bufs=1, space="PSUM") as ps,
        ):
            xsb = big.tile([128, 8 * (L + 2)], BF16, tag="xsb")
            for j in range(8):
                nc.sync.dma_start(
                    xsb[:, j * (L + 2):(j + 1) * (L + 2)],
                    xT[j * 128:(j + 1) * 128, :])
            wfsb = big.tile([128, KT_X * EL], BF16, tag="wfsb")
            for kt in range(KT_X):
                nc.sync.dma_start(wfsb[:, kt * EL:(kt + 1) * EL], wf[kt, :, :])
            bfu_t = big.tile([128, FT], F32, tag="bfu")
            nc.sync.dma_start(bfu_t[:, :], bfu[:, :])
            wbc_t = big.tile([128, FT * 32], BF16, tag="wbc")
            for ft in range(FT):
                nc.sync.dma_start(wbc_t[:, ft * 32:(ft + 1) * 32],
                                  wbc[ft, :, :])

            def body(_=None, unroll=None):
                # tc-quads: each lhsT stays stationary for 4 consecutive
                # matmuls (4x fewer PE weight loads).
                for tcQ in range(TC // 4):
                    psBC = [ps.tile([32, 512], F32, tag="bc", bufs=4,
                                    name="psBC") for _ in range(4)]
                    for ft in range(FT):
                        pts = [ps.tile([128, 512], F32, tag="acc", bufs=4,
                                       name="pt") for _ in range(4)]
                        for kt in range(KT_X):
                            kap, j = divmod(kt, 8)
                            lhsT = wfsb[:, kt * EL + ft * 128:
                                        kt * EL + (ft + 1) * 128]
                            for q in range(4):
                                t0 = (tcQ * 4 + q) * 512
                                rhs = xsb[:, j * (L + 2) + t0 + kap:
                                          j * (L + 2) + t0 + kap + 512]
                                nc.tensor.matmul(pts[q][:, :], lhsT, rhs,
                                                 start=(kt == 0),
                                                 stop=(kt == KT_X - 1))
                        for q in range(4):
                            t0 = (tcQ * 4 + q) * 512
                            ut = work.tile([128, 512], BF16, tag="u")
                            nc.scalar.activation(ut[:, :], pts[q][:, :],
                                                 AF.Silu,
                                                 bias=bfu_t[:, ft:ft + 1])
                            nc.tensor.matmul(psBC[q][:, :],
                                             wbc_t[:, ft * 32:(ft + 1) * 32],
                                             ut[:, :], start=(ft == 0),
                                             stop=(ft == FT - 1))
                            nc.sync.dma_start(
                                u_out[ft * 128:(ft + 1) * 128, t0:t0 + 512],
                                ut[:, :])
                    for q in range(4):
                        t0 = (tcQ * 4 + q) * 512
                        bco = work.tile([32, 512], F32, tag="bco", bufs=2)
                        nc.scalar.activation(bco[:, :], psBC[q][:, :],
                                             AF.Identity)
                        nc.sync.dma_start(bc_out[:, t0:t0 + 512], bco[:, :])

            if loop_reps:
                with tc.For_i(0, loop_reps, 1) as _i:
                    body()
            else:
                body()
    nc.compile()
    return nc


# ===================================================================
# Launch B: projections + scan + out-proj
# ===================================================================
def build_launch_b(loop_reps=0, pool_frac=2):
    nc = bacc.Bacc("TRN2", target_bir_lowering=False, debug=False)
    u_all = nc.dram_tensor("u_all", [KT_U, 128, L], BF16, kind="ExternalInput")
    dtw = nc.dram_tensor("dtw", [KT_U, 128, EL], BF16, kind="ExternalInput")
    dtb = nc.dram_tensor("dtb", [128, FT], F32, kind="ExternalInput")
    acol = nc.dram_tensor("acol", [128, FT * JN], F32, kind="ExternalInput")
    sel16 = nc.dram_tensor("sel16", [JN, 128, 128], BF16, kind="ExternalInput")
    sel2g = nc.dram_tensor("sel2g", [JN, 128, 128], BF16, kind="ExternalInput")
    dsk = nc.dram_tensor("dsk", [128, FT], F32, kind="ExternalInput")
    ow = nc.dram_tensor("ow", [FT, 128, 8 * 128], BF16, kind="ExternalInput")
    u_own_in = nc.dram_tensor("u_own", [FT, 128, L], BF16,
                              kind="ExternalInput")
    brep_in = nc.dram_tensor("brep", [128, L], BF16, kind="ExternalInput")
    crep_in = nc.dram_tensor("crep", [128, L], BF16, kind="ExternalInput")
    yp = nc.dram_tensor("yp", [D, L], F32, kind="ExternalOutput")

    with TileContext(nc) as tc:
        if loop_reps:
            tc.race_detector_enabled = False
        with (
            tc.tile_pool(name="big", bufs=1) as big,
            tc.tile_pool(name="work", bufs=4) as work,
            tc.tile_pool(name="ps", bufs=1, space="PSUM") as ps,
        ):
            # ---- resident weights/constants ----
            dtw_t = big.tile([128, KT_U * EL], BF16, tag="dtw")
            for kt in range(KT_U):
                nc.sync.dma_start(dtw_t[:, kt * EL:(kt + 1) * EL],
                                  dtw[kt, :, :])
            sel16_t = big.tile([128, JN * 128], BF16, tag="sel16")
            sel2g_t = big.tile([128, JN * 128], BF16, tag="sel2g")
            for g in range(JN):
                nc.sync.dma_start(sel16_t[:, g * 128:(g + 1) * 128],
                                  sel16[g, :, :])
                nc.sync.dma_start(sel2g_t[:, g * 128:(g + 1) * 128],
                                  sel2g[g, :, :])
            ow_t = big.tile([128, FT * 8 * 128], BF16, tag="ow")
            for ft in range(FT):
                nc.sync.dma_start(ow_t[:, ft * 1024:(ft + 1) * 1024],
                                  ow[ft, :, :])
            dtb_t = big.tile([128, FT], F32, tag="dtb")
            acol_t = big.tile([128, FT * JN], F32, tag="acol")
            dsk_t = big.tile([128, FT], F32, tag="dsk")
            nc.sync.dma_start(dtb_t[:, :], dtb[:, :])
            nc.sync.dma_start(acol_t[:, :], acol[:, :])
            nc.sync.dma_start(dsk_t[:, :], dsk[:, :])

            # ---- per-half resident activations ----
            delta_t = big.tile([128, FT * HL], BF16, tag="delta")
            w_t = big.tile([128, FT * HL], BF16, tag="w")
            yt_t = big.tile([128, FT * HL], BF16, tag="yt")
            hcarry = big.tile([128, FT * JN], F32, tag="hcarry")

            def body(_=None, unroll=None):
                breps = {}

                def load_bc(th):
                    t0 = th * HL
                    brep_t = work.tile([128, HL], BF16, tag="brep", bufs=2,
                                       name="brep_t")
                    crep_t = work.tile([128, HL], BF16, tag="crep", bufs=2,
                                       name="crep_t")
                    nc.sync.dma_start(brep_t[:, :], brep_in[:, t0:t0 + HL])
                    nc.sync.dma_start(crep_t[:, :], crep_in[:, t0:t0 + HL])
                    breps[th] = (brep_t, crep_t)

                def p1_ft(th, ft):
                    """dt-proj + delta + w for one (half, ft)."""
                    t0 = th * HL
                    qs = []
                    for tcI in range(HC):
                        c0 = t0 + tcI * 512
                        psP = ps.tile([128, 512], F32, tag="psP", bufs=2,
                                      name="psP")
                        for kt in range(KT_U):
                            utile = work.tile([128, 512], BF16, tag="u_in")
                            nc.sync.dma_start(utile[:, :],
                                              u_all[kt, :, c0:c0 + 512])
                            nc.tensor.matmul(
                                psP[:, :],
                                dtw_t[:, kt * EL + ft * 128:
                                      kt * EL + (ft + 1) * 128],
                                utile[:, :], start=(kt == 0),
                                stop=(kt == KT_U - 1))
                        q = work.tile([128, 512], BF16, tag="q", bufs=4)
                        nc.scalar.activation(q[:, :], psP[:, :], AF.Exp,
                                             bias=dtb_t[:, ft:ft + 1])
                        qs.append(q)
                    for tcI in range(HC):
                        l0 = tcI * 512
                        c0 = t0 + l0
                        dsl = delta_t[:, ft * HL + l0: ft * HL + l0 + 512]
                        nc.scalar.activation(dsl, qs[tcI][:, :], AF.Ln,
                                             bias=1.0)
                        uo1 = work.tile([128, 512], BF16, tag="uo1", bufs=2)
                        nc.sync.dma_start(uo1[:, :],
                                          u_own_in[ft, :, c0:c0 + 512])
                        weng = nc.gpsimd if pool_frac else nc.vector
                        weng.tensor_tensor(
                            w_t[:, ft * HL + l0: ft * HL + l0 + 512],
                            dsl, uo1[:, :], AL.mult)

                def p2_ft(th, ft):
                    t0 = th * HL
                    brep_t, crep_t = breps[th]
                    LA = 3
                    psY = [ps.tile([128, 512], F32, tag=f"psY{c}",
                                   name=f"psY{c}") for c in range(HC)]
                    chs = {}
                    hbs = {}

                    def bcast(j):
                        jg = ft * JN + j
                        selg = sel16_t[:, j * 128:(j + 1) * 128]
                        dA = work.tile([128, HL], BF16, tag="dA", bufs=3)
                        wrep = work.tile([128, HL], BF16, tag="wrep", bufs=3)
                        for c in range(HC):
                            lc = c * 512
                            psR = ps.tile([128, 512], F32,
                                          tag=f"psW{c % 2}", name="psR")
                            nc.tensor.matmul(
                                psR[:, :], selg,
                                delta_t[:, ft * HL + lc: ft * HL + lc + 512],
                                start=True, stop=True)
                            nc.scalar.activation(
                                dA[:, lc:lc + 512], psR[:, :], AF.Exp,
                                scale=acol_t[:, jg:jg + 1])
                            psW = ps.tile([128, 512], F32,
                                          tag=f"psW{(c + 1) % 2}", name="psW")
                            nc.tensor.matmul(
                                psW[:, :], selg,
                                w_t[:, ft * HL + lc: ft * HL + lc + 512],
                                start=True, stop=True)
                            nc.scalar.activation(wrep[:, lc:lc + 512],
                                                 psW[:, :], AF.Identity)
                        on_pool = pool_frac and (j % 3 != 2)
                        xeng = nc.gpsimd if on_pool else nc.vector
                        p = 1 if on_pool else 0
                        Xb = work.tile([128, HL], BF16, tag=f"Xb{p}", bufs=2)
                        xeng.tensor_tensor(Xb[:, :], wrep[:, :],
                                           brep_t[:, :], AL.mult)
                        hb = work.tile([128, HL], BF16, tag="hb", bufs=3)
                        init = 0.0 if th == 0 else hcarry[:, jg:jg + 1]
                        nc.vector.tensor_tensor_scan(hb[:, :], dA[:, :],
                                                     Xb[:, :], init,
                                                     AL.mult, AL.add)
                        hbs[j] = (hb, xeng, p, jg)

                    def chmul(j):
                        hb, xeng, p, jg = hbs.pop(j)
                        if th == 0:
                            nc.scalar.activation(hcarry[:, jg:jg + 1],
                                                 hb[:, HL - 1:HL],
                                                 AF.Identity)
                        ch = work.tile([128, HL], BF16, tag=f"ch{p}", bufs=3)
                        xeng.tensor_tensor(ch[:, :], hb[:, :],
                                           crep_t[:, :], AL.mult)
                        chs[j] = ch

                    def contract(j):
                        ch = chs.pop(j)
                        for c in range(HC):
                            nc.tensor.matmul(
                                psY[c][:, :],
                                sel2g_t[:, j * 128:(j + 1) * 128],
                                ch[:, c * 512:(c + 1) * 512],
                                start=(j == 0), stop=(j == JN - 1))

                    for j in range(JN):
                        bcast(j)
                        if j >= 1:
                            chmul(j - 1)
                        if j >= LA:
                            contract(j - LA)
                    chmul(JN - 1)
                    for j in range(JN - LA, JN):
                        contract(j)
                    uo3 = work.tile([128, HL], BF16, tag="uo3", bufs=2)
                    nc.sync.dma_start(uo3[:, :], u_own_in[ft, :, t0:t0 + HL])
                    for c in range(HC):
                        nc.vector.scalar_tensor_tensor(
                            yt_t[:, ft * HL + c * 512:
                                 ft * HL + (c + 1) * 512],
                            uo3[:, c * 512:(c + 1) * 512],
                            dsk_t[:, ft:ft + 1],
                            psY[c][:, :], AL.mult, AL.add)

                def p3(th):
                    t0 = th * HL
                    for mt in range(8):
                        for c in range(HC):
                            psO = ps.tile([128, 512], F32,
                                          tag=f"psW{c % 2}", name="psO")
                            for ft in range(FT):
                                nc.tensor.matmul(
                                    psO[:, :],
                                    ow_t[:, ft * 1024 + mt * 128:
                                         ft * 1024 + (mt + 1) * 128],
                                    yt_t[:, ft * HL + c * 512:
                                         ft * HL + (c + 1) * 512],
                                    start=(ft == 0), stop=(ft == FT - 1))
                            ot = work.tile([128, 512], F32, tag="ot", bufs=2)
                            nc.scalar.activation(ot[:, :], psO[:, :],
                                                 AF.Identity)
                            nc.sync.dma_start(
                                yp[mt * 128:(mt + 1) * 128,
                                   t0 + c * 512: t0 + (c + 1) * 512],
                                ot[:, :])

                # schedule: P1 of half th+1 (and later fts of th) interleave
                # under P2 of half th via the dedicated psP accumulator.
                load_bc(0)
                p1_ft(0, 0)
                p1_ft(0, 1)
                p2_ft(0, 0); p1_ft(0, 2)
                p2_ft(0, 1); p1_ft(0, 3)
                load_bc(1)
                p2_ft(0, 2); p1_ft(1, 0)
                p2_ft(0, 3); p1_ft(1, 1)
                p3(0)
                p2_ft(1, 0); p1_ft(1, 2)
                p2_ft(1, 1); p1_ft(1, 3)
                p2_ft(1, 2)
                p2_ft(1, 3)
                p3(1)

            if loop_reps:
                with tc.For_i(0, loop_reps, 1) as _i:
                    body()
            else:
                body()
    nc.compile()
    return nc


# ===================================================================
# Host-side weight preparation
# ===================================================================
def prepare(inputs):
    x = np.asarray(inputs["x"], np.float32)
    conv_w = np.asarray(inputs["conv_w"], np.float32)
    conv_b = np.asarray(inputs["conv_b"], np.float32)
    in_w = np.asarray(inputs["in_w"], np.float32)
    in_b = np.asarray(inputs["in_b"], np.float32)
    A_log = np.asarray(inputs["A_log"], np.float32)
    Dskip = np.asarray(inputs["Dskip"], np.float32)
    dt_w = np.asarray(inputs["dt_w"], np.float32)
    dt_b = np.asarray(inputs["dt_b"], np.float32)
    Bp_w = np.asarray(inputs["Bp_w"], np.float32)
    Bp_b = np.asarray(inputs["Bp_b"], np.float32)
    Cp_w = np.asarray(inputs["Cp_w"], np.float32)
    Cp_b = np.asarray(inputs["Cp_b"], np.float32)
    out_w = np.asarray(inputs["out_w"], np.float32)
    out_b = np.asarray(inputs["out_b"], np.float32)

    # fused conv+in_proj: Wc[f,d,k] = sum_e in_w[f,e] conv_w[e,d,k]
    Wf = (in_w @ conv_w.reshape(E, D * K)).reshape(E, D, K)
    Wf_knl = Wf.transpose(2, 1, 0).reshape(K * D, E)   # [(kap,d), f]
    b_fused = in_w @ conv_b + in_b                      # [E]

    A = -np.exp(A_log)                                  # [E, N]

    prep = {"A": A}
    prep["xT"] = []
    for b in range(B):
        xt = np.zeros((D, L + 2), np.float32)
        xt[:, 1:L + 1] = x[b].T
        prep["xT"].append(_bf16(xt))

    prep["wf"], prep["bfu"], prep["wbc"] = [], [], []
    prep["dtw"], prep["dtb"] = [], []
    prep["acol"], prep["dskc"], prep["owk"] = [], [], []
    for s in range(TPG):
        Fc = slice(s * EL, (s + 1) * EL)
        prep["wf"].append(_bf16(Wf_knl[:, Fc].reshape(KT_X, 128, EL)))
        prep["bfu"].append(
            np.ascontiguousarray(b_fused[Fc].reshape(FT, 128).T,
                                 dtype=np.float32))
        # B/C partial-projection stationary: [ft][p, 0:16]=Bp_w rows,
        # [ft][p, 16:32]=Cp_w rows for the core's channels.
        wbcs = np.empty((FT, 128, 32), np.float32)
        for ft in range(FT):
            ch = slice(s * EL + ft * 128, s * EL + (ft + 1) * 128)
            wbcs[ft, :, 0:16] = Bp_w[:, ch].T
            wbcs[ft, :, 16:32] = Cp_w[:, ch].T
        prep["wbc"].append(_bf16(wbcs))
        prep["dtw"].append(_bf16(dt_w[Fc, :].T.reshape(KT_U, 128, EL)))
        prep["dtb"].append(
            np.ascontiguousarray(dt_b[Fc].reshape(FT, 128).T,
                                 dtype=np.float32))
        ac = np.empty((128, FT * JN), np.float32)
        p = np.arange(128)
        for j in range(FT * JN):
            ac[:, j] = A[s * EL + 8 * j + (p >> 4), p & 15]
        prep["acol"].append(ac)
        prep["dskc"].append(
            np.ascontiguousarray(Dskip[Fc].reshape(FT, 128).T,
                                 dtype=np.float32))
        owk = np.empty((FT, 128, 8 * 128), np.float32)
        for ft in range(FT):
            owk[ft] = out_w[:, s * EL + ft * 128:s * EL + (ft + 1) * 128].T
        prep["owk"].append(_bf16(owk))

    prep["Bp_b"], prep["Cp_b"] = Bp_b, Cp_b

    # selectors
    sel16 = np.zeros((JN, 128, 128), np.float32)
    for g in range(JN):
        for p in range(128):
            sel16[g, 8 * g + (p >> 4), p] = 1.0
    sel2g = np.zeros((JN, 128, 128), np.float32)
    for r in range(JN):
        for k in range(128):
            sel2g[r, k, 8 * r + (k >> 4)] = 1.0
    prep["sel16_g"] = _bf16(sel16)
    prep["sel2g_g"] = _bf16(sel2g)
    prep["out_b"] = out_b
    return prep


# ===================================================================
# Orchestration
# ===================================================================
_CACHE = {}


def _get_kernels(loop_reps=0):
    key = ("k", loop_reps)
    if key not in _CACHE:
        _CACHE[key] = (build_launch_a(loop_reps), build_launch_b(loop_reps))
    return _CACHE[key]


def run_launch_a(nca, prep, **kw):
    in_maps = []
    for c in range(NCORES):
        b, s = divmod(c, TPG)
        in_maps.append(dict(xT=prep["xT"][b], wf=prep["wf"][s],
                            bfu=prep["bfu"][s], wbc=prep["wbc"][s]))
    res = bass_utils.run_bass_kernel_spmd(nca, in_maps,
                                          core_ids=list(range(NCORES)), **kw)
    return ([r["u_out"] for r in res.results],
            [r["bc_out"] for r in res.results])


def host_mid(prep, u_shards, bc_shards):
    """Gather u, reduce B/C partials, build rep-layout brep/crep."""
    u_full, breps, creps = [], [], []
    pidx = np.arange(128)
    for b in range(B):
        u_full.append(np.concatenate(u_shards[b * TPG:(b + 1) * TPG], axis=0))
        bc = np.sum([np.asarray(bc_shards[b * TPG + s], np.float32)
                     for s in range(TPG)], axis=0)       # [32, L]
        Bt = bc[0:16] + prep["Bp_b"][:, None]
        Ct = bc[16:32] + prep["Cp_b"][:, None]
        breps.append(_bf16(Bt[pidx % 16, :]))
        creps.append(_bf16(Ct[pidx % 16, :]))
    return u_full, breps, creps


def run_launch_b(ncb, prep, u_full, breps, creps, **kw):
    in_maps = []
    for c in range(NCORES):
        b, s = divmod(c, TPG)
        ub = u_full[b]                      # [E, L] bf16
        in_maps.append(dict(
            u_all=np.ascontiguousarray(ub.reshape(KT_U, 128, L)),
            u_own=np.ascontiguousarray(
                ub[s * EL:(s + 1) * EL].reshape(FT, 128, L)),
            dtw=prep["dtw"][s], dtb=prep["dtb"][s],
            acol=prep["acol"][s], sel16=prep["sel16_g"],
            sel2g=prep["sel2g_g"], dsk=prep["dskc"][s], ow=prep["owk"][s],
            brep=breps[b], crep=creps[b],
        ))
    res = bass_utils.run_bass_kernel_spmd(ncb, in_maps,
                                          core_ids=list(range(NCORES)), **kw)
    return [r["yp"] for r in res.results]


def kernel(**inputs):
    prep = prepare(inputs)
    nca, ncb = _get_kernels(_LOOP_REPS)
    u_shards, bc_shards = run_launch_a(nca, prep)
    u_full, breps, creps = host_mid(prep, u_shards, bc_shards)
    yps = run_launch_b(ncb, prep, u_full, breps, creps)
    out = np.empty((B, L, D), np.float32)
    for b in range(B):
        acc = yps[b * TPG].astype(np.float32)
        for s in range(1, TPG):
            acc = acc + yps[b * TPG + s]
        out[b] = acc.T + prep["out_b"][None, :]
    return out


# revision 3
# speedup vs baseline: 2.2061x; 1.3861x over previous
"""Trainium2 Bass kernel for the Mamba-style selective-scan block
(nn_Block_24962349924931) — v2.

Shapes: x [2, 4096, 1024]; D_MODEL=1024, D_INNER=2048, D_STATE=16, K=3.

Sharding: 8 cores = DP2 (batch) x TP4 (d_inner channels, 512/core).
Two device launches with a host-side exchange of u between them:

  Launch A: u = silu(im2col(x) @ W_fused + b_fused) per channel shard,
            plus B/C projection partials (contraction over own channels).
  host: gather u shards -> full u per batch; sum B/C partials, add bias,
        replicate rows to the (channel,state) partition layout.
  Launch B: delta = softplus(u @ dt_w^T + dt_b) (exp/ln on ACT);
            dA/w broadcast to the rep layout via selector matmuls;
            X = wrep*brep and ch = h*crep split across DVE+Pool engines;
            selective scan via tensor_tensor_scan on DVE (time halves,
            state carried in hcarry); y contract via accumulating selector
            matmuls; out_partial = (y + u*Dskip) @ out_w^T.
            P1 of the next half is interleaved into P2 of the current one
            (dedicated psP accumulator bank keeps it off P2's PSUM tags).
  host: sum the 4 TP partials per batch, add out_b, stack batches.
"""
import sys
sys.path.insert(0, "/opt/trn_rl_repo")

import numpy as np
import ml_dtypes

import concourse.bacc as bacc
import concourse.mybir as mybir
from concourse import bass_utils
from concourse.tile import TileContext

F32 = mybir.dt.float32
BF16 = mybir.dt.bfloat16
AL = mybir.AluOpType
AF = mybir.ActivationFunctionType
BF16NP = ml_dtypes.bfloat16

# ---- problem constants ----
B, L, D, E, N, K = 2, 4096, 1024, 2048, 16, 3
NCORES = 8
TPG = 4              # cores per batch (tensor parallel group)
EL = E // TPG        # 512 channels per core
FT = EL // 128       # 4 f-tiles per core
KT_U = E // 128      # 16 k-tiles over full E
KT_X = (D * K) // 128  # 24 k-tiles over im2col contraction
TC = L // 512        # 8 t-chunks
JN = 16              # rep-tiles per f-tile (8 channels x 16 states each)
TH = 2               # time halves in launch B
HL = L // TH         # 2048 cols per half
HC = HL // 512       # 4 chunks per half

_LOOP_REPS = 0


def _bf16(a):
    return np.ascontiguousarray(np.asarray(a).astype(BF16NP))


# ===================================================================
# Launch A: u = silu(im2col(x) @ W_fused + b_fused) + B/C partials
# ===================================================================
def build_launch_a(loop_reps=0):
    nc = bacc.Bacc("TRN2", target_bir_lowering=False, debug=False)
    xT = nc.dram_tensor("xT", [D, L + 2], BF16, kind="ExternalInput")
    wf = nc.dram_tensor("wf", [KT_X, 128, EL], BF16, kind="ExternalInput")
    bfu = nc.dram_tensor("bfu", [128, FT], F32, kind="ExternalInput")
    wbc = nc.dram_tensor("wbc", [FT, 128, 32], BF16, kind="ExternalInput")
    u_out = nc.dram_tensor("u_out", [EL, L], BF16, kind="ExternalOutput")
    bc_out = nc.dram_tensor("bc_out", [32, L], F32, kind="ExternalOutput")

    with TileContext(nc) as tc:
        if loop_reps:
            tc.race_detector_enabled = False
        with (
            tc.tile_pool(name="big", bufs=1) as big,
            tc.tile_pool(name="work", bufs=4) as work,
            tc.tile_pool(name="ps", bufs=1, space="PSUM") as ps,
        ):
            xsb = big.tile([128, 8 * (L + 2)], BF16, tag="xsb")
            for j in range(8):
                nc.sync.dma_start(
                    xsb[:, j * (L + 2):(j + 1) * (L + 2)],
                    xT[j * 128:(j + 1) * 128, :])
            wfsb = big.tile([128, KT_X * EL], BF16, tag="wfsb")
            for kt in range(KT_X):
                nc.sync.dma_start(wfsb[:, kt * EL:(kt + 1) * EL], wf[kt, :, :])
            bfu_t = big.tile([128, FT], F32, tag="bfu")
            nc.sync.dma_start(bfu_t[:, :], bfu[:, :])
            wbc_t = big.tile([128, FT * 32], BF16, tag="wbc")
            for ft in range(FT):
                nc.sync.dma_start(wbc_t[:, ft * 32:(ft + 1) * 32],
                                  wbc[ft, :, :])

            def body(_=None, unroll=None):
                for tcI in range(TC):
                    t0 = tcI * 512
                    psBC = ps.tile([32, 512], F32, tag="bc", bufs=2,
                                   name="psBC")
                    for ft in range(FT):
                        pt = ps.tile([128, 512], F32, tag="acc", bufs=4,
                                     name="pt")
                        for kt in range(KT_X):
                            kap, j = divmod(kt, 8)
                            rhs = xsb[:, j * (L + 2) + t0 + kap:
                                      j * (L + 2) + t0 + kap + 512]
                            lhsT = wfsb[:, kt * EL + ft * 128:
                                        kt * EL + (ft + 1) * 128]
                            nc.tensor.matmul(pt[:, :], lhsT, rhs,
                                             start=(kt == 0),
                                             stop=(kt == KT_X - 1))
                        ut = work.tile([128, 512], BF16, tag="u")
                        nc.scalar.activation(ut[:, :], pt[:, :], AF.Silu,
                                             bias=bfu_t[:, ft:ft + 1])
                        nc.tensor.matmul(psBC[:, :],
                                         wbc_t[:, ft * 32:(ft + 1) * 32],
                                         ut[:, :], start=(ft == 0),
                                         stop=(ft == FT - 1))
                        nc.sync.dma_start(
                            u_out[ft * 128:(ft + 1) * 128, t0:t0 + 512],
                            ut[:, :])
                    bco = work.tile([32, 512], F32, tag="bco", bufs=2)
                    nc.scalar.activation(bco[:, :], psBC[:, :], AF.Identity)
                    nc.sync.dma_start(bc_out[:, t0:t0 + 512], bco[:, :])

            if loop_reps:
                with tc.For_i(0, loop_reps, 1) as _i:
                    body()
            else:
                body()
    nc.compile()
    return nc


# ===================================================================
# Launch B: projections + scan + out-proj
# ===================================================================
def build_launch_b(loop_reps=0, pool_frac=2):
    nc = bacc.Bacc("TRN2", target_bir_lowering=False, debug=False)
    u_all = nc.dram_tensor("u_all", [KT_U, 128, L], BF16, kind="ExternalInput")
    dtw = nc.dram_tensor("dtw", [KT_U, 128, EL], BF16, kind="ExternalInput")
    dtb = nc.dram_tensor("dtb", [128, FT], F32, kind="ExternalInput")
    acol = nc.dram_tensor("acol", [128, FT * JN], F32, kind="ExternalInput")
    sel16 = nc.dram_tensor("sel16", [JN, 128, 128], BF16, kind="ExternalInput")
    sel2g = nc.dram_tensor("sel2g", [JN, 128, 128], BF16, kind="ExternalInput")
    dsk = nc.dram_tensor("dsk", [128, FT], F32, kind="ExternalInput")
    ow = nc.dram_tensor("ow", [FT, 128, 8 * 128], BF16, kind="ExternalInput")
    u_own_in = nc.dram_tensor("u_own", [FT, 128, L], BF16,
                              kind="ExternalInput")
    brep_in = nc.dram_tensor("brep", [128, L], BF16, kind="ExternalInput")
    crep_in = nc.dram_tensor("crep", [128, L], BF16, kind="ExternalInput")
    yp = nc.dram_tensor("yp", [D, L], F32, kind="ExternalOutput")

    with TileContext(nc) as tc:
        if loop_reps:
            tc.race_detector_enabled = False
        with (
            tc.tile_pool(name="big", bufs=1) as big,
            tc.tile_pool(name="work", bufs=4) as work,
            tc.tile_pool(name="ps", bufs=1, space="PSUM") as ps,
        ):
            # ---- resident weights/constants ----
            dtw_t = big.tile([128, KT_U * EL], BF16, tag="dtw")
            for kt in range(KT_U):
                nc.sync.dma_start(dtw_t[:, kt * EL:(kt + 1) * EL],
                                  dtw[kt, :, :])
            sel16_t = big.tile([128, JN * 128], BF16, tag="sel16")
            sel2g_t = big.tile([128, JN * 128], BF16, tag="sel2g")
            for g in range(JN):
                nc.sync.dma_start(sel16_t[:, g * 128:(g + 1) * 128],
                                  sel16[g, :, :])
                nc.sync.dma_start(sel2g_t[:, g * 128:(g + 1) * 128],
                                  sel2g[g, :, :])
            ow_t = big.tile([128, FT * 8 * 128], BF16, tag="ow")
            for ft in range(FT):
                nc.sync.dma_start(ow_t[:, ft * 1024:(ft + 1) * 1024],
                                  ow[ft, :, :])
            dtb_t = big.tile([128, FT], F32, tag="dtb")
            acol_t = big.tile([128, FT * JN], F32, tag="acol")
            dsk_t = big.tile([128, FT], F32, tag="dsk")
            nc.sync.dma_start(dtb_t[:, :], dtb[:, :])
            nc.sync.dma_start(acol_t[:, :], acol[:, :])
            nc.sync.dma_start(dsk_t[:, :], dsk[:, :])

            # ---- per-half resident activations ----
            delta_t = big.tile([128, FT * HL], BF16, tag="delta")
            w_t = big.tile([128, FT * HL], BF16, tag="w")
            yt_t = big.tile([128, FT * HL], BF16, tag="yt")
            hcarry = big.tile([128, FT * JN], F32, tag="hcarry")

            def body(_=None, unroll=None):
                breps = {}

                def load_bc(th):
                    t0 = th * HL
                    brep_t = work.tile([128, HL], BF16, tag="brep", bufs=2,
                                       name="brep_t")
                    crep_t = work.tile([128, HL], BF16, tag="crep", bufs=2,
                                       name="crep_t")
                    nc.sync.dma_start(brep_t[:, :], brep_in[:, t0:t0 + HL])
                    nc.sync.dma_start(crep_t[:, :], crep_in[:, t0:t0 + HL])
                    breps[th] = (brep_t, crep_t)

                def p1_ft(th, ft):
                    """dt-proj + delta + w for one (half, ft)."""
                    t0 = th * HL
                    qs = []
                    for tcI in range(HC):
                        c0 = t0 + tcI * 512
                        psP = ps.tile([128, 512], F32, tag="psP", bufs=2,
                                      name="psP")
                        for kt in range(KT_U):
                            utile = work.tile([128, 512], BF16, tag="u_in")
                            nc.sync.dma_start(utile[:, :],
                                              u_all[kt, :, c0:c0 + 512])
                            nc.tensor.matmul(
                                psP[:, :],
                                dtw_t[:, kt * EL + ft * 128:
                                      kt * EL + (ft + 1) * 128],
                                utile[:, :], start=(kt == 0),
                                stop=(kt == KT_U - 1))
                        q = work.tile([128, 512], BF16, tag="q", bufs=4)
                        nc.scalar.activation(q[:, :], psP[:, :], AF.Exp,
                                             bias=dtb_t[:, ft:ft + 1])
                        qs.append(q)
                    for tcI in range(HC):
                        l0 = tcI * 512
                        c0 = t0 + l0
                        dsl = delta_t[:, ft * HL + l0: ft * HL + l0 + 512]
                        nc.scalar.activation(dsl, qs[tcI][:, :], AF.Ln,
                                             bias=1.0)
                        uo1 = work.tile([128, 512], BF16, tag="uo1", bufs=2)
                        nc.sync.dma_start(uo1[:, :],
                                          u_own_in[ft, :, c0:c0 + 512])
                        weng = nc.gpsimd if pool_frac else nc.vector
                        weng.tensor_tensor(
                            w_t[:, ft * HL + l0: ft * HL + l0 + 512],
                            dsl, uo1[:, :], AL.mult)

                def p2_ft(th, ft):
                    t0 = th * HL
                    brep_t, crep_t = breps[th]
                    LA = 3
                    psY = [ps.tile([128, 512], F32, tag=f"psY{c}",
                                   name=f"psY{c}") for c in range(HC)]
                    chs = {}
                    hbs = {}

                    def bcast(j):
                        jg = ft * JN + j
                        selg = sel16_t[:, j * 128:(j + 1) * 128]
                        dA = work.tile([128, HL], BF16, tag="dA", bufs=3)
                        wrep = work.tile([128, HL], BF16, tag="wrep", bufs=3)
                        for c in range(HC):
                            lc = c * 512
                            psR = ps.tile([128, 512], F32,
                                          tag=f"psW{c % 2}", name="psR")
                            nc.tensor.matmul(
                                psR[:, :], selg,
                                delta_t[:, ft * HL + lc: ft * HL + lc + 512],
                                start=True, stop=True)
                            nc.scalar.activation(
                                dA[:, lc:lc + 512], psR[:, :], AF.Exp,
                                scale=acol_t[:, jg:jg + 1])
                            psW = ps.tile([128, 512], F32,
                                          tag=f"psW{(c + 1) % 2}", name="psW")
                            nc.tensor.matmul(
                                psW[:, :], selg,
                                w_t[:, ft * HL + lc: ft * HL + lc + 512],
                                start=True, stop=True)
                            nc.scalar.activation(wrep[:, lc:lc + 512],
                                                 psW[:, :], AF.Identity)
                        on_pool = pool_frac and (j % 3 != 2)
                        xeng = nc.gpsimd if on_pool else nc.vector
                        p = 1 if on_pool else 0
                        Xb = work.tile([128, HL], BF16, tag=f"Xb{p}", bufs=2)
                        xeng.tensor_tensor(Xb[:, :], wrep[:, :],
                                           brep_t[:, :], AL.mult)
                        hb = work.tile([128, HL], BF16, tag="hb", bufs=3)
                        init = 0.0 if th == 0 else hcarry[:, jg:jg + 1]
                        nc.vector.tensor_tensor_scan(hb[:, :], dA[:, :],
                                                     Xb[:, :], init,
                                                     AL.mult, AL.add)
                        hbs[j] = (hb, xeng, p, jg)

                    def chmul(j):
                        hb, xeng, p, jg = hbs.pop(j)
                        if th == 0:
                            nc.scalar.activation(hcarry[:, jg:jg + 1],
                                                 hb[:, HL - 1:HL],
                                                 AF.Identity)
                        ch = work.tile([128, HL], BF16, tag=f"ch{p}", bufs=3)
                        xeng.tensor_tensor(ch[:, :], hb[:, :],
                                           crep_t[:, :], AL.mult)
                        chs[j] = ch

                    def contract(j):
                        ch = chs.pop(j)
                        for c in range(HC):
                            nc.tensor.matmul(
                                psY[c][:, :],
                                sel2g_t[:, j * 128:(j + 1) * 128],
                                ch[:, c * 512:(c + 1) * 512],
                                start=(j == 0), stop=(j == JN - 1))

                    for j in range(JN):
                        bcast(j)
                        if j >= 1:
                            chmul(j - 1)
                        if j >= LA:
                            contract(j - LA)
                    chmul(JN - 1)
                    for j in range(JN - LA, JN):
                        contract(j)
                    uo3 = work.tile([128, HL], BF16, tag="uo3", bufs=2)
                    nc.sync.dma_start(uo3[:, :], u_own_in[ft, :, t0:t0 + HL])
                    for c in range(HC):
                        nc.vector.scalar_tensor_tensor(
                            yt_t[:, ft * HL + c * 512:
                                 ft * HL + (c + 1) * 512],
                            uo3[:, c * 512:(c + 1) * 512],
                            dsk_t[:, ft:ft + 1],
                            psY[c][:, :], AL.mult, AL.add)

                def p3(th):
                    t0 = th * HL
                    for mt in range(8):
                        for c in range(HC):
                            psO = ps.tile([128, 512], F32,
                                          tag=f"psW{c % 2}", name="psO")
                            for ft in range(FT):
                                nc.tensor.matmul(
                                    psO[:, :],
                                    ow_t[:, ft * 1024 + mt * 128:
                                         ft * 1024 + (mt + 1) * 128],
                                    yt_t[:, ft * HL + c * 512:
                                         ft * HL + (c + 1) * 512],
                                    start=(ft == 0), stop=(ft == FT - 1))
                            ot = work.tile([128, 512], F32, tag="ot", bufs=2)
                            nc.scalar.activation(ot[:, :], psO[:, :],
                                                 AF.Identity)
                            nc.sync.dma_start(
                                yp[mt * 128:(mt + 1) * 128,
                                   t0 + c * 512: t0 + (c + 1) * 512],
                                ot[:, :])

                # schedule: P1 of half th+1 (and later fts of th) interleave
                # under P2 of half th via the dedicated psP accumulator.
                load_bc(0)
                p1_ft(0, 0)
                p1_ft(0, 1)
                p2_ft(0, 0); p1_ft(0, 2)
                p2_ft(0, 1); p1_ft(0, 3)
                load_bc(1)
                p2_ft(0, 2); p1_ft(1, 0)
                p2_ft(0, 3); p1_ft(1, 1)
                p3(0)
                p2_ft(1, 0); p1_ft(1, 2)
                p2_ft(1, 1); p1_ft(1, 3)
                p2_ft(1, 2)
                p2_ft(1, 3)
                p3(1)

            if loop_reps:
                with tc.For_i(0, loop_reps, 1) as _i:
                    body()
            else:
                body()
    nc.compile()
    return nc


# ===================================================================
# Host-side weight preparation
# ===================================================================
def prepare(inputs):
    x = np.asarray(inputs["x"], np.float32)
    conv_w = np.asarray(inputs["conv_w"], np.float32)
    conv_b = np.asarray(inputs["conv_b"], np.float32)
    in_w = np.asarray(inputs["in_w"], np.float32)
    in_b = np.asarray(inputs["in_b"], np.float32)
    A_log = np.asarray(inputs["A_log"], np.float32)
    Dskip = np.asarray(inputs["Dskip"], np.float32)
    dt_w = np.asarray(inputs["dt_w"], np.float32)
    dt_b = np.asarray(inputs["dt_b"], np.float32)
    Bp_w = np.asarray(inputs["Bp_w"], np.float32)
    Bp_b = np.asarray(inputs["Bp_b"], np.float32)
    Cp_w = np.asarray(inputs["Cp_w"], np.float32)
    Cp_b = np.asarray(inputs["Cp_b"], np.float32)
    out_w = np.asarray(inputs["out_w"], np.float32)
    out_b = np.asarray(inputs["out_b"], np.float32)

    # fused conv+in_proj: Wc[f,d,k] = sum_e in_w[f,e] conv_w[e,d,k]
    Wf = (in_w @ conv_w.reshape(E, D * K)).reshape(E, D, K)
    Wf_knl = Wf.transpose(2, 1, 0).reshape(K * D, E)   # [(kap,d), f]
    b_fused = in_w @ conv_b + in_b                      # [E]

    A = -np.exp(A_log)                                  # [E, N]

    prep = {"A": A}
    prep["xT"] = []
    for b in range(B):
        xt = np.zeros((D, L + 2), np.float32)
        xt[:, 1:L + 1] = x[b].T
        prep["xT"].append(_bf16(xt))

    prep["wf"], prep["bfu"], prep["wbc"] = [], [], []
    prep["dtw"], prep["dtb"] = [], []
    prep["acol"], prep["dskc"], prep["owk"] = [], [], []
    for s in range(TPG):
        Fc = slice(s * EL, (s + 1) * EL)
        prep["wf"].append(_bf16(Wf_knl[:, Fc].reshape(KT_X, 128, EL)))
        prep["bfu"].append(
            np.ascontiguousarray(b_fused[Fc].reshape(FT, 128).T,
                                 dtype=np.float32))
        # B/C partial-projection stationary: [ft][p, 0:16]=Bp_w rows,
        # [ft][p, 16:32]=Cp_w rows for the core's channels.
        wbcs = np.empty((FT, 128, 32), np.float32)
        for ft in range(FT):
            ch = slice(s * EL + ft * 128, s * EL + (ft + 1) * 128)
            wbcs[ft, :, 0:16] = Bp_w[:, ch].T
            wbcs[ft, :, 16:32] = Cp_w[:, ch].T
        prep["wbc"].append(_bf16(wbcs))
        prep["dtw"].append(_bf16(dt_w[Fc, :].T.reshape(KT_U, 128, EL)))
        prep["dtb"].append(
            np.ascontiguousarray(dt_b[Fc].reshape(FT, 128).T,
                                 dtype=np.float32))
        ac = np.empty((128, FT * JN), np.float32)
        p = np.arange(128)
        for j in range(FT * JN):
            ac[:, j] = A[s * EL + 8 * j + (p >> 4), p & 15]
        prep["acol"].append(ac)
        prep["dskc"].append(
            np.ascontiguousarray(Dskip[Fc].reshape(FT, 128).T,
                                 dtype=np.float32))
        owk = np.empty((FT, 128, 8 * 128), np.float32)
        for ft in range(FT):
            owk[ft] = out_w[:, s * EL + ft * 128:s * EL + (ft + 1) * 128].T
        prep["owk"].append(_bf16(owk))

    prep["Bp_b"], prep["Cp_b"] = Bp_b, Cp_b

    # selectors
    sel16 = np.zeros((JN, 128, 128), np.float32)
    for g in range(JN):
        for p in range(128):
            sel16[g, 8 * g + (p >> 4), p] = 1.0
    sel2g = np.zeros((JN, 128, 128), np.float32)
    for r in range(JN):
        for k in range(128):
            sel2g[r, k, 8 * r + (k >> 4)] = 1.0
    prep["sel16_g"] = _bf16(sel16)
    prep["sel2g_g"] = _bf16(sel2g)
    prep["out_b"] = out_b
    return prep


# ===================================================================
# Orchestration
# ===================================================================
_CACHE = {}


def _get_kernels(loop_reps=0):
    key = ("k", loop_reps)
    if key not in _CACHE:
        _CACHE[key] = (build_launch_a(loop_reps), build_launch_b(loop_reps))
    return _CACHE[key]


def run_launch_a(nca, prep, **kw):
    in_maps = []
    for c in range(NCORES):
        b, s = divmod(c, TPG)
        in_maps.append(dict(xT=prep["xT"][b], wf=prep["wf"][s],
                            bfu=prep["bfu"][s], wbc=prep["wbc"][s]))
    res = bass_utils.run_bass_kernel_spmd(nca, in_maps,
                                          core_ids=list(range(NCORES)), **kw)
    return ([r["u_out"] for r in res.results],
            [r["bc_out"] for r in res.results])


def host_mid(prep, u_shards, bc_shards):
    """Gather u, reduce B/C partials, build rep-layout brep/crep."""
    u_full, breps, creps = [], [], []
    pidx = np.arange(128)
    for b in range(B):
        u_full.append(np.concatenate(u_shards[b * TPG:(b + 1) * TPG], axis=0))
        bc = np.sum([np.asarray(bc_shards[b * TPG + s], np.float32)
                     for s in range(TPG)], axis=0)       # [32, L]
        Bt = bc[0:16] + prep["Bp_b"][:, None]
        Ct = bc[16:32] + prep["Cp_b"][:, None]
        breps.append(_bf16(Bt[pidx % 16, :]))
        creps.append(_bf16(Ct[pidx % 16, :]))
    return u_full, breps, creps


def run_launch_b(ncb, prep, u_full, breps, creps, **kw):
    in_maps = []
    for c in range(NCORES):
        b, s = divmod(c, TPG)
        ub = u_full[b]                      # [E, L] bf16
        in_maps.append(dict(
            u_all=np.ascontiguousarray(ub.reshape(KT_U, 128, L)),
            u_own=np.ascontiguousarray(
                ub[s * EL:(s + 1) * EL].reshape(FT, 128, L)),
            dtw=prep["dtw"][s], dtb=prep["dtb"][s],
            acol=prep["acol"][s], sel16=prep["sel16_g"],
            sel2g=prep["sel2g_g"], dsk=prep["dskc"][s], ow=prep["owk"][s],
            brep=breps[b], crep=creps[b],
        ))
    res = bass_utils.run_bass_kernel_spmd(ncb, in_maps,
                                          core_ids=list(range(NCORES)), **kw)
    return [r["yp"] for r in res.results]


def kernel(**inputs):
    prep = prepare(inputs)
    nca, ncb = _get_kernels(_LOOP_REPS)
    u_shards, bc_shards = run_launch_a(nca, prep)
    u_full, breps, creps = host_mid(prep, u_shards, bc_shards)
    yps = run_launch_b(ncb, prep, u_full, breps, creps)
    out = np.empty((B, L, D), np.float32)
    for b in range(B):
        acc = yps[b * TPG].astype(np.float32)
        for s in range(1, TPG):
            acc = acc + yps[b * TPG + s]
        out[b] = acc.T + prep["out_b"][None, :]
    return out


# revision 4
# speedup vs baseline: 2.2190x; 1.0058x over previous
"""Trainium2 Bass kernel for the Mamba-style selective-scan block
(nn_Block_24962349924931) — v2.

Shapes: x [2, 4096, 1024]; D_MODEL=1024, D_INNER=2048, D_STATE=16, K=3.

Sharding: 8 cores = DP2 (batch) x TP4 (d_inner channels, 512/core).
Two device launches with a host-side exchange of u between them:

  Launch A: u = silu(im2col(x) @ W_fused + b_fused) per channel shard,
            plus B/C projection partials (contraction over own channels).
  host: gather u shards -> full u per batch; sum B/C partials, add bias,
        replicate rows to the (channel,state) partition layout.
  Launch B: delta = softplus(u @ dt_w^T + dt_b) (exp/ln on ACT);
            dA/w broadcast to the rep layout via selector matmuls;
            X = wrep*brep and ch = h*crep split across DVE+Pool engines;
            selective scan via tensor_tensor_scan on DVE (time halves,
            state carried in hcarry); y contract via accumulating selector
            matmuls; out_partial = (y + u*Dskip) @ out_w^T.
            P1 of the next half is interleaved into P2 of the current one
            (dedicated psP accumulator bank keeps it off P2's PSUM tags).
  host: sum the 4 TP partials per batch, add out_b, stack batches.
"""
import sys
sys.path.insert(0, "/opt/trn_rl_repo")

import numpy as np
import ml_dtypes

import concourse.bacc as bacc
import concourse.mybir as mybir
from concourse import bass_utils
from concourse.tile import TileContext

F32 = mybir.dt.float32
BF16 = mybir.dt.bfloat16
AL = mybir.AluOpType
AF = mybir.ActivationFunctionType
BF16NP = ml_dtypes.bfloat16

# ---- problem constants ----
B, L, D, E, N, K = 2, 4096, 1024, 2048, 16, 3
NCORES = 8
TPG = 4              # cores per batch (tensor parallel group)
EL = E // TPG        # 512 channels per core
FT = EL // 128       # 4 f-tiles per core
KT_U = E // 128      # 16 k-tiles over full E
KT_X = (D * K) // 128  # 24 k-tiles over im2col contraction
TC = L // 512        # 8 t-chunks
JN = 16              # rep-tiles per f-tile (8 channels x 16 states each)
TH = 2               # time halves in launch B
HL = L // TH         # 2048 cols per half
HC = HL // 512       # 4 chunks per half

_LOOP_REPS = 0


def _bf16(a):
    return np.ascontiguousarray(np.asarray(a).astype(BF16NP))


# ===================================================================
# Launch A: u = silu(im2col(x) @ W_fused + b_fused) + B/C partials
# ===================================================================
def build_launch_a(loop_reps=0):
    nc = bacc.Bacc("TRN2", target_bir_lowering=False, debug=False)
    xT = nc.dram_tensor("xT", [D, L + 2], BF16, kind="ExternalInput")
    wf = nc.dram_tensor("wf", [KT_X, 128, EL], BF16, kind="ExternalInput")
    bfu = nc.dram_tensor("bfu", [128, FT], F32, kind="ExternalInput")
    wbc = nc.dram_tensor("wbc", [FT, 128, 32], BF16, kind="ExternalInput")
    u_out = nc.dram_tensor("u_out", [EL, L], BF16, kind="ExternalOutput")
    bc_out = nc.dram_tensor("bc_out", [32, L], F32, kind="ExternalOutput")

    with TileContext(nc) as tc:
        if loop_reps:
            tc.race_detector_enabled = False
        with (
            tc.tile_pool(name="big", bufs=1) as big,
            tc.tile_pool(name="work", bufs=4) as work,
            tc.tile_pool(name="ps", bufs=1, space="PSUM") as ps,
        ):
            xsb = big.tile([128, 8 * (L + 2)], BF16, tag="xsb")
            for j in range(8):
                nc.sync.dma_start(
                    xsb[:, j * (L + 2):(j + 1) * (L + 2)],
                    xT[j * 128:(j + 1) * 128, :])
            wfsb = big.tile([128, KT_X * EL], BF16, tag="wfsb")
            for kt in range(KT_X):
                nc.sync.dma_start(wfsb[:, kt * EL:(kt + 1) * EL], wf[kt, :, :])
            bfu_t = big.tile([128, FT], F32, tag="bfu")
            nc.sync.dma_start(bfu_t[:, :], bfu[:, :])
            wbc_t = big.tile([128, FT * 32], BF16, tag="wbc")
            for ft in range(FT):
                nc.sync.dma_start(wbc_t[:, ft * 32:(ft + 1) * 32],
                                  wbc[ft, :, :])

            def body(_=None, unroll=None):
                for tcI in range(TC):
                    t0 = tcI * 512
                    psBC = ps.tile([32, 512], F32, tag="bc", bufs=2,
                                   name="psBC")
                    for ft in range(FT):
                        pt = ps.tile([128, 512], F32, tag="acc", bufs=4,
                                     name="pt")
                        for kt in range(KT_X):
                            kap, j = divmod(kt, 8)
                            rhs = xsb[:, j * (L + 2) + t0 + kap:
                                      j * (L + 2) + t0 + kap + 512]
                            lhsT = wfsb[:, kt * EL + ft * 128:
                                        kt * EL + (ft + 1) * 128]
                            nc.tensor.matmul(pt[:, :], lhsT, rhs,
                                             start=(kt == 0),
                                             stop=(kt == KT_X - 1))
                        ut = work.tile([128, 512], BF16, tag="u")
                        nc.scalar.activation(ut[:, :], pt[:, :], AF.Silu,
                                             bias=bfu_t[:, ft:ft + 1])
                        nc.tensor.matmul(psBC[:, :],
                                         wbc_t[:, ft * 32:(ft + 1) * 32],
                                         ut[:, :], start=(ft == 0),
                                         stop=(ft == FT - 1))
                        nc.sync.dma_start(
                            u_out[ft * 128:(ft + 1) * 128, t0:t0 + 512],
                            ut[:, :])
                    bco = work.tile([32, 512], F32, tag="bco", bufs=2)
                    nc.scalar.activation(bco[:, :], psBC[:, :], AF.Identity)
                    nc.sync.dma_start(bc_out[:, t0:t0 + 512], bco[:, :])

            if loop_reps:
                with tc.For_i(0, loop_reps, 1) as _i:
                    body()
            else:
                body()
    nc.compile()
    return nc


# ===================================================================
# Launch B: projections + scan + out-proj
# ===================================================================
def build_launch_b(loop_reps=0, pool_frac=0, la=3, abufs=3, poolmod=3, poolrem=2, xpool=True, chpool=True):
    nc = bacc.Bacc("TRN2", target_bir_lowering=False, debug=False)
    u_all = nc.dram_tensor("u_all", [KT_U, 128, L], BF16, kind="ExternalInput")
    dtw = nc.dram_tensor("dtw", [KT_U, 128, EL], BF16, kind="ExternalInput")
    dtb = nc.dram_tensor("dtb", [128, FT], F32, kind="ExternalInput")
    acol = nc.dram_tensor("acol", [128, FT * JN], F32, kind="ExternalInput")
    sel16 = nc.dram_tensor("sel16", [JN, 128, 128], BF16, kind="ExternalInput")
    sel2g = nc.dram_tensor("sel2g", [JN, 128, 128], BF16, kind="ExternalInput")
    dsk = nc.dram_tensor("dsk", [128, FT], F32, kind="ExternalInput")
    ow = nc.dram_tensor("ow", [FT, 128, 8 * 128], BF16, kind="ExternalInput")
    u_own_in = nc.dram_tensor("u_own", [FT, 128, L], BF16,
                              kind="ExternalInput")
    brep_in = nc.dram_tensor("brep", [128, L], BF16, kind="ExternalInput")
    crep_in = nc.dram_tensor("crep", [128, L], BF16, kind="ExternalInput")
    yp = nc.dram_tensor("yp", [D, L], F32, kind="ExternalOutput")

    with TileContext(nc) as tc:
        if loop_reps:
            tc.race_detector_enabled = False
        with (
            tc.tile_pool(name="big", bufs=1) as big,
            tc.tile_pool(name="work", bufs=4) as work,
            tc.tile_pool(name="ps", bufs=1, space="PSUM") as ps,
        ):
            # ---- resident weights/constants ----
            dtw_t = big.tile([128, KT_U * EL], BF16, tag="dtw")
            for kt in range(KT_U):
                nc.sync.dma_start(dtw_t[:, kt * EL:(kt + 1) * EL],
                                  dtw[kt, :, :])
            sel16_t = big.tile([128, JN * 128], BF16, tag="sel16")
            sel2g_t = big.tile([128, JN * 128], BF16, tag="sel2g")
            for g in range(JN):
                nc.sync.dma_start(sel16_t[:, g * 128:(g + 1) * 128],
                                  sel16[g, :, :])
                nc.sync.dma_start(sel2g_t[:, g * 128:(g + 1) * 128],
                                  sel2g[g, :, :])
            ow_t = big.tile([128, FT * 8 * 128], BF16, tag="ow")
            for ft in range(FT):
                nc.sync.dma_start(ow_t[:, ft * 1024:(ft + 1) * 1024],
                                  ow[ft, :, :])
            dtb_t = big.tile([128, FT], F32, tag="dtb")
            acol_t = big.tile([128, FT * JN], F32, tag="acol")
            dsk_t = big.tile([128, FT], F32, tag="dsk")
            nc.sync.dma_start(dtb_t[:, :], dtb[:, :])
            nc.sync.dma_start(acol_t[:, :], acol[:, :])
            nc.sync.dma_start(dsk_t[:, :], dsk[:, :])

            # ---- per-half resident activations ----
            delta_t = big.tile([128, FT * HL], BF16, tag="delta")
            w_t = big.tile([128, FT * HL], BF16, tag="w")
            yt_t = big.tile([128, FT * HL], BF16, tag="yt")
            hcarry = big.tile([128, FT * JN], F32, tag="hcarry")

            def body(_=None, unroll=None):
                breps = {}

                def load_bc(th):
                    t0 = th * HL
                    brep_t = work.tile([128, HL], BF16, tag="brep", bufs=2,
                                       name="brep_t")
                    crep_t = work.tile([128, HL], BF16, tag="crep", bufs=2,
                                       name="crep_t")
                    nc.sync.dma_start(brep_t[:, :], brep_in[:, t0:t0 + HL])
                    nc.sync.dma_start(crep_t[:, :], crep_in[:, t0:t0 + HL])
                    breps[th] = (brep_t, crep_t)

                def p1_ft(th, ft):
                    """dt-proj + delta + w for one (half, ft)."""
                    t0 = th * HL
                    qs = []
                    for tcI in range(HC):
                        c0 = t0 + tcI * 512
                        psP = ps.tile([128, 512], F32, tag="psP", bufs=2,
                                      name="psP")
                        for kt in range(KT_U):
                            utile = work.tile([128, 512], BF16, tag="u_in")
                            nc.sync.dma_start(utile[:, :],
                                              u_all[kt, :, c0:c0 + 512])
                            nc.tensor.matmul(
                                psP[:, :],
                                dtw_t[:, kt * EL + ft * 128:
                                      kt * EL + (ft + 1) * 128],
                                utile[:, :], start=(kt == 0),
                                stop=(kt == KT_U - 1))
                        q = work.tile([128, 512], BF16, tag="q", bufs=4)
                        nc.scalar.activation(q[:, :], psP[:, :], AF.Exp,
                                             bias=dtb_t[:, ft:ft + 1])
                        qs.append(q)
                    for tcI in range(HC):
                        l0 = tcI * 512
                        c0 = t0 + l0
                        dsl = delta_t[:, ft * HL + l0: ft * HL + l0 + 512]
                        nc.scalar.activation(dsl, qs[tcI][:, :], AF.Ln,
                                             bias=1.0)
                        uo1 = work.tile([128, 512], BF16, tag="uo1", bufs=2)
                        nc.sync.dma_start(uo1[:, :],
                                          u_own_in[ft, :, c0:c0 + 512])
                        weng = nc.gpsimd if pool_frac else nc.vector
                        weng.tensor_tensor(
                            w_t[:, ft * HL + l0: ft * HL + l0 + 512],
                            dsl, uo1[:, :], AL.mult)

                def p2_ft(th, ft):
                    t0 = th * HL
                    brep_t, crep_t = breps[th]
                    LA = la
                    psY = [ps.tile([128, 512], F32, tag=f"psY{c}",
                                   name=f"psY{c}") for c in range(HC)]
                    chs = {}
                    hbs = {}

                    def bcast(j):
                        jg = ft * JN + j
                        selg = sel16_t[:, j * 128:(j + 1) * 128]
                        dA = work.tile([128, HL], BF16, tag="dA", bufs=abufs)
                        wrep = work.tile([128, HL], BF16, tag="wrep", bufs=abufs)
                        for c in range(HC):
                            lc = c * 512
                            psR = ps.tile([128, 512], F32,
                                          tag=f"psW{c % 2}", name="psR")
                            nc.tensor.matmul(
                                psR[:, :], selg,
                                delta_t[:, ft * HL + lc: ft * HL + lc + 512],
                                start=True, stop=True)
                            nc.scalar.activation(
                                dA[:, lc:lc + 512], psR[:, :], AF.Exp,
                                scale=acol_t[:, jg:jg + 1])
                            psW = ps.tile([128, 512], F32,
                                          tag=f"psW{(c + 1) % 2}", name="psW")
                            nc.tensor.matmul(
                                psW[:, :], selg,
                                w_t[:, ft * HL + lc: ft * HL + lc + 512],
                                start=True, stop=True)
                            nc.scalar.activation(wrep[:, lc:lc + 512],
                                                 psW[:, :], AF.Identity)
                        on_pool = pool_frac and (j % poolmod != poolrem)
                        xon = on_pool and xpool
                        xeng = nc.gpsimd if xon else nc.vector
                        xp_ = 1 if xon else 0
                        Xb = work.tile([128, HL], BF16, tag=f"Xb{xp_}",
                                       bufs=2)
                        xeng.tensor_tensor(Xb[:, :], wrep[:, :],
                                           brep_t[:, :], AL.mult)
                        p = 1 if (on_pool and chpool) else 0
                        xeng = nc.gpsimd if (on_pool and chpool) else nc.vector
                        hb = work.tile([128, HL], BF16, tag="hb", bufs=3)
                        init = 0.0 if th == 0 else hcarry[:, jg:jg + 1]
                        nc.vector.tensor_tensor_scan(hb[:, :], dA[:, :],
                                                     Xb[:, :], init,
                                                     AL.mult, AL.add)
                        hbs[j] = (hb, xeng, p, jg)

                    def chmul(j):
                        hb, xeng, p, jg = hbs.pop(j)
                        if th == 0:
                            nc.scalar.activation(hcarry[:, jg:jg + 1],
                                                 hb[:, HL - 1:HL],
                                                 AF.Identity)
                        ch = work.tile([128, HL], BF16, tag=f"ch{p}", bufs=3)
                        xeng.tensor_tensor(ch[:, :], hb[:, :],
                                           crep_t[:, :], AL.mult)
                        chs[j] = ch

                    def contract(j):
                        ch = chs.pop(j)
                        for c in range(HC):
                            nc.tensor.matmul(
                                psY[c][:, :],
                                sel2g_t[:, j * 128:(j + 1) * 128],
                                ch[:, c * 512:(c + 1) * 512],
                                start=(j == 0), stop=(j == JN - 1))

                    for j in range(JN):
                        bcast(j)
                        if j >= 1:
                            chmul(j - 1)
                        if j >= LA:
                            contract(j - LA)
                    chmul(JN - 1)
                    for j in range(JN - LA, JN):
                        contract(j)
                    uo3 = work.tile([128, HL], BF16, tag="uo3", bufs=2)
                    nc.sync.dma_start(uo3[:, :], u_own_in[ft, :, t0:t0 + HL])
                    for c in range(HC):
                        nc.vector.scalar_tensor_tensor(
                            yt_t[:, ft * HL + c * 512:
                                 ft * HL + (c + 1) * 512],
                            uo3[:, c * 512:(c + 1) * 512],
                            dsk_t[:, ft:ft + 1],
                            psY[c][:, :], AL.mult, AL.add)

                def p3(th):
                    t0 = th * HL
                    for mt in range(8):
                        for c in range(HC):
                            psO = ps.tile([128, 512], F32,
                                          tag=f"psW{c % 2}", name="psO")
                            for ft in range(FT):
                                nc.tensor.matmul(
                                    psO[:, :],
                                    ow_t[:, ft * 1024 + mt * 128:
                                         ft * 1024 + (mt + 1) * 128],
                                    yt_t[:, ft * HL + c * 512:
                                         ft * HL + (c + 1) * 512],
                                    start=(ft == 0), stop=(ft == FT - 1))
                            ot = work.tile([128, 512], F32, tag="ot", bufs=2)
                            nc.scalar.activation(ot[:, :], psO[:, :],
                                                 AF.Identity)
                            nc.sync.dma_start(
                                yp[mt * 128:(mt + 1) * 128,
                                   t0 + c * 512: t0 + (c + 1) * 512],
                                ot[:, :])

                # schedule: P1 of half th+1 (and later fts of th) interleave
                # under P2 of half th via the dedicated psP accumulator.
                load_bc(0)
                p1_ft(0, 0)
                p1_ft(0, 1)
                p2_ft(0, 0); p1_ft(0, 2)
                p2_ft(0, 1); p1_ft(0, 3)
                load_bc(1)
                p2_ft(0, 2); p1_ft(1, 0)
                p2_ft(0, 3); p1_ft(1, 1)
                p3(0)
                p2_ft(1, 0); p1_ft(1, 2)
                p2_ft(1, 1); p1_ft(1, 3)
                p2_ft(1, 2)
                p2_ft(1, 3)
                p3(1)

            if loop_reps:
                with tc.For_i(0, loop_reps, 1) as _i:
                    body()
            else:
                body()
    nc.compile()
    return nc


# ===================================================================
# Host-side weight preparation
# ===================================================================
def prepare(inputs):
    x = np.asarray(inputs["x"], np.float32)
    conv_w = np.asarray(inputs["conv_w"], np.float32)
    conv_b = np.asarray(inputs["conv_b"], np.float32)
    in_w = np.asarray(inputs["in_w"], np.float32)
    in_b = np.asarray(inputs["in_b"], np.float32)
    A_log = np.asarray(inputs["A_log"], np.float32)
    Dskip = np.asarray(inputs["Dskip"], np.float32)
    dt_w = np.asarray(inputs["dt_w"], np.float32)
    dt_b = np.asarray(inputs["dt_b"], np.float32)
    Bp_w = np.asarray(inputs["Bp_w"], np.float32)
    Bp_b = np.asarray(inputs["Bp_b"], np.float32)
    Cp_w = np.asarray(inputs["Cp_w"], np.float32)
    Cp_b = np.asarray(inputs["Cp_b"], np.float32)
    out_w = np.asarray(inputs["out_w"], np.float32)
    out_b = np.asarray(inputs["out_b"], np.float32)

    # fused conv+in_proj: Wc[f,d,k] = sum_e in_w[f,e] conv_w[e,d,k]
    Wf = (in_w @ conv_w.reshape(E, D * K)).reshape(E, D, K)
    Wf_knl = Wf.transpose(2, 1, 0).reshape(K * D, E)   # [(kap,d), f]
    b_fused = in_w @ conv_b + in_b                      # [E]

    A = -np.exp(A_log)                                  # [E, N]

    prep = {"A": A}
    prep["xT"] = []
    for b in range(B):
        xt = np.zeros((D, L + 2), np.float32)
        xt[:, 1:L + 1] = x[b].T
        prep["xT"].append(_bf16(xt))

    prep["wf"], prep["bfu"], prep["wbc"] = [], [], []
    prep["dtw"], prep["dtb"] = [], []
    prep["acol"], prep["dskc"], prep["owk"] = [], [], []
    for s in range(TPG):
        Fc = slice(s * EL, (s + 1) * EL)
        prep["wf"].append(_bf16(Wf_knl[:, Fc].reshape(KT_X, 128, EL)))
        prep["bfu"].append(
            np.ascontiguousarray(b_fused[Fc].reshape(FT, 128).T,
                                 dtype=np.float32))
        # B/C partial-projection stationary: [ft][p, 0:16]=Bp_w rows,
        # [ft][p, 16:32]=Cp_w rows for the core's channels.
        wbcs = np.empty((FT, 128, 32), np.float32)
        for ft in range(FT):
            ch = slice(s * EL + ft * 128, s * EL + (ft + 1) * 128)
            wbcs[ft, :, 0:16] = Bp_w[:, ch].T
            wbcs[ft, :, 16:32] = Cp_w[:, ch].T
        prep["wbc"].append(_bf16(wbcs))
        prep["dtw"].append(_bf16(dt_w[Fc, :].T.reshape(KT_U, 128, EL)))
        prep["dtb"].append(
            np.ascontiguousarray(dt_b[Fc].reshape(FT, 128).T,
                                 dtype=np.float32))
        ac = np.empty((128, FT * JN), np.float32)
        p = np.arange(128)
        for j in range(FT * JN):
            ac[:, j] = A[s * EL + 8 * j + (p >> 4), p & 15]
        prep["acol"].append(ac)
        prep["dskc"].append(
            np.ascontiguousarray(Dskip[Fc].reshape(FT, 128).T,
                                 dtype=np.float32))
        owk = np.empty((FT, 128, 8 * 128), np.float32)
        for ft in range(FT):
            owk[ft] = out_w[:, s * EL + ft * 128:s * EL + (ft + 1) * 128].T
        prep["owk"].append(_bf16(owk))

    prep["Bp_b"], prep["Cp_b"] = Bp_b, Cp_b

    # selectors
    sel16 = np.zeros((JN, 128, 128), np.float32)
    for g in range(JN):
        for p in range(128):
            sel16[g, 8 * g + (p >> 4), p] = 1.0
    sel2g = np.zeros((JN, 128, 128), np.float32)
    for r in range(JN):
        for k in range(128):
            sel2g[r, k, 8 * r + (k >> 4)] = 1.0
    prep["sel16_g"] = _bf16(sel16)
    prep["sel2g_g"] = _bf16(sel2g)
    prep["out_b"] = out_b
    return prep


# ===================================================================
# Orchestration
# ===================================================================
_CACHE = {}


def _get_kernels(loop_reps=0):
    key = ("k", loop_reps)
    if key not in _CACHE:
        _CACHE[key] = (build_launch_a(loop_reps), build_launch_b(loop_reps))
    return _CACHE[key]


def run_launch_a(nca, prep, **kw):
    in_maps = []
    for c in range(NCORES):
        b, s = divmod(c, TPG)
        in_maps.append(dict(xT=prep["xT"][b], wf=prep["wf"][s],
                            bfu=prep["bfu"][s], wbc=prep["wbc"][s]))
    res = bass_utils.run_bass_kernel_spmd(nca, in_maps,
                                          core_ids=list(range(NCORES)), **kw)
    return ([r["u_out"] for r in res.results],
            [r["bc_out"] for r in res.results])


def host_mid(prep, u_shards, bc_shards):
    """Gather u, reduce B/C partials, build rep-layout brep/crep."""
    u_full, breps, creps = [], [], []
    pidx = np.arange(128)
    for b in range(B):
        u_full.append(np.concatenate(u_shards[b * TPG:(b + 1) * TPG], axis=0))
        bc = np.sum([np.asarray(bc_shards[b * TPG + s], np.float32)
                     for s in range(TPG)], axis=0)       # [32, L]
        Bt = bc[0:16] + prep["Bp_b"][:, None]
        Ct = bc[16:32] + prep["Cp_b"][:, None]
        breps.append(_bf16(Bt[pidx % 16, :]))
        creps.append(_bf16(Ct[pidx % 16, :]))
    return u_full, breps, creps


def run_launch_b(ncb, prep, u_full, breps, creps, **kw):
    in_maps = []
    for c in range(NCORES):
        b, s = divmod(c, TPG)
        ub = u_full[b]                      # [E, L] bf16
        in_maps.append(dict(
            u_all=np.ascontiguousarray(ub.reshape(KT_U, 128, L)),
            u_own=np.ascontiguousarray(
                ub[s * EL:(s + 1) * EL].reshape(FT, 128, L)),
            dtw=prep["dtw"][s], dtb=prep["dtb"][s],
            acol=prep["acol"][s], sel16=prep["sel16_g"],
            sel2g=prep["sel2g_g"], dsk=prep["dskc"][s], ow=prep["owk"][s],
            brep=breps[b], crep=creps[b],
        ))
    res = bass_utils.run_bass_kernel_spmd(ncb, in_maps,
                                          core_ids=list(range(NCORES)), **kw)
    return [r["yp"] for r in res.results]


def kernel(**inputs):
    prep = prepare(inputs)
    nca, ncb = _get_kernels(_LOOP_REPS)
    u_shards, bc_shards = run_launch_a(nca, prep)
    u_full, breps, creps = host_mid(prep, u_shards, bc_shards)
    yps = run_launch_b(ncb, prep, u_full, breps, creps)
    out = np.empty((B, L, D), np.float32)
    for b in range(B):
        acc = yps[b * TPG].astype(np.float32)
        for s in range(1, TPG):
            acc = acc + yps[b * TPG + s]
        out[b] = acc.T + prep["out_b"][None, :]
    return out
